# revision 10
# baseline (speedup 1.0000x reference)
"""DiffGLCM Trainium2 kernel.

Reference computes t = diff(sigmoid one-hot) per pixel, then
GLCM = sum_p t_center(p) outer t_periph(p), normalized per image.

We use linearity: t = E a where a = [1, sigma_1..sigma_63] and E is the
bidiagonal difference matrix (t_j = a_j - a_{j+1}, a_64 := 0). So
GLCM = E (sum_p a_c a_p^T) E^T. The kernel computes the raw sigmoid
co-occurrence S = sum_p a_c a_p^T on the PE (a generated on-chip with
one DVE subtract + one ACT sigmoid per tile; the constant-1 row comes
from sigmoid(640*(x+10)) == 1.0 exactly), and the host applies the E
transforms (a 2D second difference) and per-image normalization.

Batch of 16 images is sharded 2-per-core across 8 NeuronCores.
"""

import sys

sys.path.insert(0, "/opt/trn_rl_repo")

import numpy as np

import concourse.bass as bass
import concourse.mybir as mybir
import concourse.tile as tile
from concourse.bass_utils import run_bass_kernel_spmd

F32 = mybir.dt.float32
H = W = 256
NIMG = 2          # images per core
NG = 64           # grey levels; also rows of the a-vector (row 0 = const 1)
COLS = 255        # center/periph columns per strip
# (center row offset, periph row offset, rows) — periph = center + (1,1)
STRIPS = [(0, 1, 128), (128, 129, 127)]
COL_BATCHES = [(0, 64), (64, 64), (128, 64), (192, 63)]
N_MM = 2 * COLS   # matmuls per image


def _build_program():
    nc = bass.Bass()
    xs = nc.declare_dram_parameter("xs", [NIMG, H, W], F32, isOutput=False)
    shift = nc.declare_dram_parameter("shift", [128, NG * 64], F32, isOutput=False)
    out = nc.declare_dram_parameter("glcm", [NIMG, NG, NG], F32, isOutput=True)

    with tile.TileContext(nc) as tc:
        with (
            tc.tile_pool(name="const", bufs=1) as const_pool,
            tc.tile_pool(name="strips", bufs=2) as strip_pool,
            tc.tile_pool(name="sig", bufs=4) as a_pool,
            tc.tile_pool(name="oub", bufs=2) as out_pool,
            tc.tile_pool(name="ps", bufs=2, space="PSUM") as psum_pool,
        ):
            # shift replicated over columns; absorbed into SBUF via one
            # copy so downstream consumers never wait on the DMA queues.
            shift_raw = const_pool.tile([128, NG, 64], F32)
            nc.sync.dma_start(
                shift_raw[:].rearrange("p a b -> p (a b)"), shift[:]
            )
            sh2 = const_pool.tile([128, NG, 64], F32)
            nc.vector.tensor_copy(
                sh2[:].rearrange("p a b -> p (a b)"),
                shift_raw[:].rearrange("p a b -> p (a b)"),
            )

            for img in range(NIMG):
                psum = psum_pool.tile([NG, NG], F32)
                mm = 0
                for r0c, r0p, P in STRIPS:
                    svc = {}
                    for nm, r0, cc in (("c", r0c, 0), ("p", r0p, 1)):
                        s = strip_pool.tile([128, COLS], F32, tag="s" + nm)
                        nc.sync.dma_start(s[:P], xs[img, r0 : r0 + P, cc : cc + COLS])
                        s2 = strip_pool.tile([128, COLS], F32, tag="s2" + nm)
                        nc.vector.tensor_copy(s2[:P], s[:P])
                        svc[nm] = s2
                    for c0, CB in COL_BATCHES:
                        aa = {}
                        for nm in ("c", "p"):
                            A = a_pool.tile([128, NG, 64], F32, tag="A" + nm)
                            xb = (
                                svc[nm][:P, c0 : c0 + CB]
                                .unsqueeze(1)
                                .broadcast_to([P, NG, CB])
                            )
                            # A = x - shift  (shift[0] = -10 -> const-1 row)
                            nc.vector.tensor_sub(
                                A[:P, :, 0:CB], xb, sh2[:P, :, 0:CB]
                            )
                            # A = sigmoid(640 * A)
                            nc.scalar.activation(
                                A[:P, :, 0:CB],
                                A[:P, :, 0:CB],
                                mybir.ActivationFunctionType.Sigmoid,
                                scale=640.0,
                            )
                            aa[nm] = A
                        for c in range(CB):
                            nc.tensor.matmul(
                                psum[:, :],
                                aa["c"][:P, :, c],
                                aa["p"][:P, :, c],
                                start=(mm == 0),
                                stop=(mm == N_MM - 1),
                            )
                            mm += 1
                ob = out_pool.tile([NG, NG], F32)
                nc.vector.tensor_copy(ob[:], psum[:, :])
                nc.sync.dma_start(out[img], ob[:])
    _split_waits(nc)
    return nc


def _split_waits(nc):
    """This walrus build rejects >1 sync wait on ANY instruction struct
    (even Tile's own end-of-kernel drain). Rewrite every multi-wait
    instruction into a chain of single-wait same-engine drains followed
    by the instruction carrying its last wait.
    """
    n = 0
    for bb in nc.m.functions[0].blocks:
        out = []
        for ins in bb.instructions:
            si = ins.sync_info
            if si is not None and si.on_wait and len(si.on_wait) > 1:
                waits = list(si.on_wait)
                for w in waits[:-1]:
                    out.append(
                        mybir.InstDrain(
                            name=f"waitsplit-{n}",
                            engine=ins.engine,
                            sync_info=mybir.SyncInfo(on_wait=[w], on_update=[]),
                        )
                    )
                    n += 1
                ins.sync_info = mybir.SyncInfo(
                    on_wait=waits[-1:], on_update=list(si.on_update or [])
                )
            out.append(ins)
        bb.instructions[:] = out
    return n


def make_in_maps(x):
    # shift[k] = k/64 for k=1..63; slot 0 = -10 so sigmoid(640*(x+10)) == 1
    sv = np.arange(0, NG, dtype=np.float32) / np.float32(NG)
    sv[0] = -10.0
    shift = np.ascontiguousarray(
        np.broadcast_to(
            np.repeat(sv, 64)[None, :], (128, NG * 64)
        )
    )
    return [
        {"xs": np.ascontiguousarray(x[2 * k : 2 * k + 2]), "shift": shift}
        for k in range(8)
    ]


def _finish_host(s_aug):
    # G = E S E^T: t_j = a_j - a_{j+1} (a_64 = 0) on both axes, then
    # per-image normalization.
    B = s_aug.shape[0]
    p = np.zeros((B, NG + 1, NG + 1), dtype=np.float64)
    p[:, :NG, :NG] = s_aug.astype(np.float64)
    g = p[:, :NG, :NG] - p[:, 1:, :NG] - p[:, :NG, 1:] + p[:, 1:, 1:]
    g = g / g.sum(axis=(1, 2), keepdims=True)
    return g.astype(np.float32)


_NC = None


def kernel(x, offset_r=1, offset_c=1, **_):
    global _NC
    assert int(offset_r) == 1 and int(offset_c) == 1
    x = np.ascontiguousarray(np.asarray(x, dtype=np.float32).reshape(16, H, W))
    if _NC is None:
        _NC = _build_program()
    res = run_bass_kernel_spmd(_NC, make_in_maps(x), core_ids=list(range(8)))
    s_aug = np.concatenate([r["glcm"] for r in res.results], axis=0)  # [16,64,64]
    return _finish_host(s_aug).reshape(16, 1, NG, NG, 1)


if __name__ == "__main__":
    _build_program()
    print("build OK")


# revision 17
# speedup vs baseline: 1971.4748x; 1971.4748x over previous
"""DiffGLCM Trainium2 kernel.

Per image: soft one-hot t[pixel, 64] generated on-chip (DVE expand-sub,
ACT sigmoid -> bf16, DVE shifted difference; the constant-1 top row of
the accumulator comes from sigmoid(640*(x+10)) == 1 exactly, and bin 63
needs no upper edge). GLCM = sum_p t_c(p) outer t_p(p) runs as K-tiled
PE matmuls (bf16 weights, fp32 PSUM) using two concurrent 64-column PE
array tiles; the two PSUM halves are summed on host, which also does
per-image normalization. Batch of 16 images -> 2 per NeuronCore x 8.
"""

import sys

sys.path.insert(0, "/opt/trn_rl_repo")

import numpy as np

import concourse.bass as bass
import concourse.mybir as mybir
import concourse.tile as tile
from concourse.bass_utils import run_bass_kernel_spmd

F32 = mybir.dt.float32
BF16 = mybir.dt.bfloat16
H = W = 256
NIMG = 2          # images per core
NG = 64           # grey levels
COLS = 255        # center/periph columns per strip
# (center row offset, periph row offset, rows) — periph = center + (1,1)
STRIPS = [(0, 1, 128), (128, 129, 127)]
COL_BATCHES = [(0, 64), (64, 64), (128, 64), (192, 63)]


def _build_program(split=True, mm_dtype=BF16, col_tiles=2, loop_reps=0):
    import contextlib

    nc = bass.Bass()
    xs = nc.declare_dram_parameter("xs", [NIMG, H, W], F32, isOutput=False)
    shift = nc.declare_dram_parameter("shift", [128, NG * 64], F32, isOutput=False)
    out = nc.declare_dram_parameter(
        "glcm", [NIMG, 64 * col_tiles, NG], F32, isOutput=True
    )

    with tile.TileContext(nc) as tc:
        with (
            tc.tile_pool(name="const", bufs=1) as const_pool,
            tc.tile_pool(name="strips", bufs=2) as strip_pool,
            tc.tile_pool(name="sig", bufs=3) as a_pool,
            tc.tile_pool(name="th", bufs=3) as t_pool,
            tc.tile_pool(name="oub", bufs=2) as out_pool,
            tc.tile_pool(name="ps", bufs=2, space="PSUM") as psum_pool,
        ):
            # shift replicated over columns; absorbed into SBUF via one
            # copy so downstream consumers never wait on the DMA queues.
            shift_raw = const_pool.tile([128, NG, 64], F32)
            nc.sync.dma_start(
                shift_raw[:].rearrange("p a b -> p (a b)"), shift[:]
            )
            sh2 = const_pool.tile([128, NG, 64], F32)
            nc.vector.tensor_copy(
                sh2[:].rearrange("p a b -> p (a b)"),
                shift_raw[:].rearrange("p a b -> p (a b)"),
            )

            rep_ctx = (
                tc.For_i(0, loop_reps, 1) if loop_reps else contextlib.nullcontext()
            )
            with rep_ctx:
              for img in range(NIMG):
                # one PSUM bank per col-tile group; group g accumulates in
                # partitions [64g, 64g+64) to match its tile_position
                psums = []
                for g in range(col_tiles):
                    pst = psum_pool.tile([128, NG], F32, tag=f"ps{g}", name=f"ps{g}")
                    psums.append(pst)
                n_mm = 2 * COLS
                mm = 0
                for r0c, r0p, P in STRIPS:
                    svc = {}
                    for nm, r0, cc in (("c", r0c, 0), ("p", r0p, 1)):
                        s = strip_pool.tile([128, COLS], F32, tag="s" + nm)
                        nc.sync.dma_start(s[:P], xs[img, r0 : r0 + P, cc : cc + COLS])
                        s2 = strip_pool.tile([128, COLS], F32, tag="s2" + nm)
                        nc.vector.tensor_copy(s2[:P], s[:P])
                        svc[nm] = s2
                    for c0, CB in COL_BATCHES:
                        tt = {}
                        for nm in ("c", "p"):
                            A = a_pool.tile([128, NG, 64], F32, tag="A" + nm)
                            xb = (
                                svc[nm][:P, c0 : c0 + CB]
                                .unsqueeze(1)
                                .broadcast_to([P, NG, CB])
                            )
                            # A = x - shift  (shift[0] = -10 -> const-1 row)
                            nc.vector.tensor_sub(
                                A[:P, :, 0:CB], xb, sh2[:P, :, 0:CB]
                            )
                            # A = sigmoid(640 * A): sigma_k rows, row 0 == 1
                            nc.scalar.activation(
                                A[:P, :, 0:CB],
                                A[:P, :, 0:CB],
                                mybir.ActivationFunctionType.Sigmoid,
                                scale=640.0,
                            )
                            # t_j = A_j - A_{j+1} (j<63); t_63 = sigma_63
                            t = t_pool.tile([128, NG, 64], mm_dtype, tag="t" + nm)
                            nc.vector.tensor_sub(
                                t[:P, 0:63, 0:CB],
                                A[:P, 0:63, 0:CB],
                                A[:P, 1:64, 0:CB],
                            )
                            nc.vector.tensor_copy(
                                t[:P, 63, 0:CB], A[:P, 63, 0:CB]
                            )
                            tt[nm] = t
                        for c in range(CB):
                            g = mm % col_tiles
                            nc.tensor.matmul(
                                psums[g][64 * g : 64 * g + NG, :],
                                tt["c"][:P, :, c],
                                tt["p"][:P, :, c],
                                start=(mm < col_tiles),
                                stop=(mm >= n_mm - col_tiles),
                                tile_position=(0, 64 * g) if col_tiles > 1 else None,
                            )
                            mm += 1
                ob = out_pool.tile([64 * col_tiles, NG], F32)
                for g in range(col_tiles):
                    nc.vector.tensor_copy(
                        ob[64 * g : 64 * g + NG, :],
                        psums[g][64 * g : 64 * g + NG, :],
                    )
                nc.sync.dma_start(out[img], ob[:])
    if split:
        _split_waits(nc)
    return nc


def _split_waits(nc):
    """This walrus build rejects >1 sync wait on ANY instruction struct
    (even Tile's own end-of-kernel drain). Rewrite every multi-wait
    instruction into a chain of single-wait same-engine drains followed
    by the instruction carrying its last wait.
    """
    n = 0
    for bb in nc.m.functions[0].blocks:
        out = []
        for ins in bb.instructions:
            si = ins.sync_info
            if si is not None and si.on_wait and len(si.on_wait) > 1:
                waits = list(si.on_wait)
                for w in waits[:-1]:
                    out.append(
                        mybir.InstDrain(
                            name=f"waitsplit-{n}",
                            engine=ins.engine,
                            sync_info=mybir.SyncInfo(on_wait=[w], on_update=[]),
                        )
                    )
                    n += 1
                ins.sync_info = mybir.SyncInfo(
                    on_wait=waits[-1:], on_update=list(si.on_update or [])
                )
            out.append(ins)
        bb.instructions[:] = out
    return n


def make_in_maps(x):
    # shift[k] = k/64 for k=1..63; slot 0 = -10 so sigmoid(640*(x+10)) == 1
    sv = np.arange(0, NG, dtype=np.float32) / np.float32(NG)
    sv[0] = -10.0
    shift = np.ascontiguousarray(
        np.broadcast_to(np.repeat(sv, 64)[None, :], (128, NG * 64))
    )
    return [
        {"xs": np.ascontiguousarray(x[2 * k : 2 * k + 2]), "shift": shift}
        for k in range(8)
    ]


def _finish_host(raw):
    # raw: [16, 64*col_tiles, 64] — sum col-tile halves, normalize per image
    B = raw.shape[0]
    g = raw.reshape(B, -1, NG, NG).sum(axis=1, dtype=np.float64)
    g = g / g.sum(axis=(1, 2), keepdims=True)
    return g.astype(np.float32)


_NC = None


def kernel(x, offset_r=1, offset_c=1, **_):
    global _NC
    assert int(offset_r) == 1 and int(offset_c) == 1
    x = np.ascontiguousarray(np.asarray(x, dtype=np.float32).reshape(16, H, W))
    if _NC is None:
        _NC = _build_program()
    res = run_bass_kernel_spmd(_NC, make_in_maps(x), core_ids=list(range(8)))
    raw = np.concatenate([r["glcm"] for r in res.results], axis=0)
    return _finish_host(raw).reshape(16, 1, NG, NG, 1)


if __name__ == "__main__":
    _build_program()
    print("build OK")


# revision 22
# speedup vs baseline: 2833.6666x; 1.4373x over previous
"""DiffGLCM Trainium2 kernel.

Reference: t_j = A_j - A_{j+1} per pixel with A = [1, sigma_1..sigma_63, 0],
GLCM = sum_p t_c(p) outer t_p(p), normalized per image.

Kernel computes S = sum_p A_c(p) outer A_p(p) (65x65, raw sigmoid
co-occurrence) on the PE; the difference is linear, so on host
G[i,j] = S[i,j] - S[i+1,j] - S[i,j+1] + S[i+1,j+1]. The whole A vector
comes from ONE activation op: row 0 = sigmoid(640(x+10)) == 1 exactly,
rows 1..63 = bin edges, row 64 = sigmoid(640(x-11)) == 0 exactly.
Per-pixel-chunk matmuls (K<=128 pixel rows, M=N=65) accumulate in fp32
PSUM, split into 4 accumulators (2 strips x 2 parity groups) to reduce
fp32 accumulation error of the large raw sums; host sums them in fp64.
Batch of 16 images -> 2 per NeuronCore x 8 cores.
"""

import sys

sys.path.insert(0, "/opt/trn_rl_repo")

import numpy as np

import concourse.bass as bass
import concourse.mybir as mybir
import concourse.tile as tile
from concourse.bass_utils import run_bass_kernel_spmd

F32 = mybir.dt.float32
BF16 = mybir.dt.bfloat16
H = W = 256
NIMG = 2          # images per core
NG = 64           # grey levels
NR = NG + 1       # A rows: const-1, 63 edges, const-0
COLS = 255        # center/periph columns per strip
# (center row offset, periph row offset, rows) — periph = center + (1,1)
STRIPS = [(0, 1, 128), (128, 129, 127)]
COL_BATCHES = [(0, 64), (64, 64), (128, 64), (192, 63)]
N_ACC = 4         # PSUM accumulators per image


def _build_program(split=True, mm_dtype=BF16, loop_reps=0):
    import contextlib

    nc = bass.Bass()
    xs = nc.declare_dram_parameter("xs", [NIMG, H, W], F32, isOutput=False)
    shift = nc.declare_dram_parameter("shift", [128, NR * 64], F32, isOutput=False)
    out = nc.declare_dram_parameter("glcm", [NIMG, N_ACC, NR, NR], F32, isOutput=True)

    with tile.TileContext(nc) as tc:
        with (
            tc.tile_pool(name="const", bufs=1) as const_pool,
            tc.tile_pool(name="strips", bufs=2) as strip_pool,
            tc.tile_pool(name="arg", bufs=3) as arg_pool,
            tc.tile_pool(name="sig", bufs=4) as sig_pool,
            tc.tile_pool(name="oub", bufs=2) as out_pool,
            tc.tile_pool(name="ps", bufs=2, space="PSUM") as psum_pool,
        ):
            # shift replicated over columns; absorbed into SBUF via one
            # copy so downstream consumers never wait on the DMA queues.
            shift_raw = const_pool.tile([128, NR, 64], F32)
            nc.sync.dma_start(
                shift_raw[:].rearrange("p a b -> p (a b)"), shift[:]
            )
            sh2 = const_pool.tile([128, NR, 64], F32)
            nc.vector.tensor_copy(
                sh2[:].rearrange("p a b -> p (a b)"),
                shift_raw[:].rearrange("p a b -> p (a b)"),
            )

            rep_ctx = (
                tc.For_i(0, loop_reps, 1) if loop_reps else contextlib.nullcontext()
            )
            with rep_ctx:
              for img in range(NIMG):
                psums = []
                for g in range(N_ACC):
                    pst = psum_pool.tile([NR, NR], F32, tag=f"ps{g}", name=f"ps{g}")
                    psums.append(pst)
                # matmuls per accumulator: strip s parity q -> 255 cols split
                acc_mm = [0] * N_ACC
                acc_total = [128, 127, 128, 127]  # ceil/floor of 255 by parity

                for si, (r0c, r0p, P) in enumerate(STRIPS):
                    svc = {}
                    for nm, r0, cc in (("c", r0c, 0), ("p", r0p, 1)):
                        s = strip_pool.tile(
                            [128, COLS], F32, tag="s" + nm, name="s" + nm
                        )
                        nc.sync.dma_start(s[:P], xs[img, r0 : r0 + P, cc : cc + COLS])
                        s2 = strip_pool.tile(
                            [128, COLS], F32, tag="s2" + nm, name="s2" + nm
                        )
                        nc.vector.tensor_copy(s2[:P], s[:P])
                        svc[nm] = s2
                    for c0, CB in COL_BATCHES:
                        sgs = {}
                        for nm in ("c", "p"):
                            A = arg_pool.tile([128, NR, 64], F32, tag="arg", name="A")
                            xb = (
                                svc[nm][:P, c0 : c0 + CB]
                                .unsqueeze(1)
                                .broadcast_to([P, NR, CB])
                            )
                            # A = x - shift: row 0 -> x+10, row 64 -> x-11
                            nc.vector.tensor_sub(A[:P, :, 0:CB], xb, sh2[:P, :, 0:CB])
                            # sig = sigmoid(640*A); rows 0/64 exactly 1/0
                            sg = sig_pool.tile(
                                [128, NR, 64], mm_dtype, tag="sg" + nm, name="sg" + nm
                            )
                            nc.scalar.activation(
                                sg[:P, :, 0:CB],
                                A[:P, :, 0:CB],
                                mybir.ActivationFunctionType.Sigmoid,
                                scale=640.0,
                            )
                            sgs[nm] = sg
                        for c in range(CB):
                            acc = 2 * si + ((c0 + c) % 2)
                            nc.tensor.matmul(
                                psums[acc][:, :],
                                sgs["c"][:P, :, c],
                                sgs["p"][:P, :, c],
                                start=(acc_mm[acc] == 0),
                                stop=(acc_mm[acc] == acc_total[acc] - 1),
                            )
                            acc_mm[acc] += 1
                # ob: [65 partitions, N_ACC, 65] — each psum copied to one slot
                ob = out_pool.tile([NR, N_ACC, NR], F32, name="ob")
                for g in range(N_ACC):
                    nc.vector.tensor_copy(ob[:, g, :], psums[g][:, :])
                nc.sync.dma_start(
                    out[img].rearrange("a r c -> r a c"), ob[:]
                )
    if split:
        _split_waits(nc)
    return nc


def _split_waits(nc):
    """This walrus build rejects >1 sync wait on ANY instruction struct
    (even Tile's own end-of-kernel drain). Rewrite every multi-wait
    instruction into a chain of single-wait same-engine drains followed
    by the instruction carrying its last wait.
    """
    n = 0
    for bb in nc.m.functions[0].blocks:
        out = []
        for ins in bb.instructions:
            si = ins.sync_info
            if si is not None and si.on_wait and len(si.on_wait) > 1:
                waits = list(si.on_wait)
                for w in waits[:-1]:
                    out.append(
                        mybir.InstDrain(
                            name=f"waitsplit-{n}",
                            engine=ins.engine,
                            sync_info=mybir.SyncInfo(on_wait=[w], on_update=[]),
                        )
                    )
                    n += 1
                ins.sync_info = mybir.SyncInfo(
                    on_wait=waits[-1:], on_update=list(si.on_update or [])
                )
            out.append(ins)
        bb.instructions[:] = out
    return n


def make_in_maps(x):
    # shift[0] = -10 (sigmoid == 1), shift[k] = k/64, shift[64] = +11
    # (sigmoid == 0 for x in [0,1))
    sv = np.arange(0, NR, dtype=np.float32) / np.float32(NG)
    sv[0] = -10.0
    sv[NG] = 11.0
    shift = np.ascontiguousarray(
        np.broadcast_to(np.repeat(sv, 64)[None, :], (128, NR * 64))
    )
    return [
        {"xs": np.ascontiguousarray(x[2 * k : 2 * k + 2]), "shift": shift}
        for k in range(8)
    ]


def _finish_host(raw):
    # raw: [16, N_ACC, NR, NR] — fp64-sum accumulators, 2D second
    # difference (the E transform on both axes), then normalize.
    s = raw.astype(np.float64).sum(axis=1)  # [16, NR, NR]
    g = s[:, :NG, :NG] - s[:, 1:, :NG] - s[:, :NG, 1:] + s[:, 1:, 1:]
    g = g / g.sum(axis=(1, 2), keepdims=True)
    return g.astype(np.float32)


_NC = None


def kernel(x, offset_r=1, offset_c=1, **_):
    global _NC
    assert int(offset_r) == 1 and int(offset_c) == 1
    x = np.ascontiguousarray(np.asarray(x, dtype=np.float32).reshape(16, H, W))
    if _NC is None:
        _NC = _build_program()
    res = run_bass_kernel_spmd(_NC, make_in_maps(x), core_ids=list(range(8)))
    raw = np.concatenate([r["glcm"] for r in res.results], axis=0)
    return _finish_host(raw).reshape(16, 1, NG, NG, 1)


if __name__ == "__main__":
    _build_program()
    print("build OK")


# revision 26
# speedup vs baseline: 2960.5884x; 1.0448x over previous
"""DiffGLCM Trainium2 kernel.

Reference: t_j = A_j - A_{j+1} per pixel with A = [1, sigma_1..sigma_63, 0],
GLCM = sum_p t_c(p) outer t_p(p), normalized per image.

Kernel computes S = sum_p A_c(p) outer A_p(p) (65x65, raw sigmoid
co-occurrence) on the PE; the difference is linear, so on host
G[i,j] = S[i,j] - S[i+1,j] - S[i,j+1] + S[i+1,j+1]. The whole A vector
comes from ONE activation op: row 0 = sigmoid(640(x+10)) == 1 exactly,
rows 1..63 = bin edges, row 64 = sigmoid(640(x-11)) == 0 exactly.
Per-pixel-chunk matmuls (K<=128 pixel rows, M=N=65) accumulate in fp32
PSUM, split into 4 accumulators (2 strips x 2 parity groups) to reduce
fp32 accumulation error of the large raw sums; host sums them in fp64.
Batch of 16 images -> 2 per NeuronCore x 8 cores.
"""

import sys

sys.path.insert(0, "/opt/trn_rl_repo")

import numpy as np

import concourse.bass as bass
import concourse.mybir as mybir
import concourse.tile as tile
from concourse.bass_utils import run_bass_kernel_spmd

F32 = mybir.dt.float32
BF16 = mybir.dt.bfloat16
H = W = 256
NIMG = 2          # images per core
NG = 64           # grey levels
NR = NG + 1       # A rows: const-1, 63 edges, const-0
COLS = 255        # center/periph columns per strip
# (center row offset, periph row offset, rows) — periph = center + (1,1)
STRIPS = [(0, 1, 128), (128, 129, 127)]
COL_BATCHES = [(0, 64), (64, 64), (128, 64), (192, 63)]
N_ACC = 4         # PSUM accumulators per image


def _build_program(split=True, mm_dtype=BF16, loop_reps=0):
    import contextlib

    nc = bass.Bass()
    xs = nc.declare_dram_parameter("xs", [NIMG, H, W], F32, isOutput=False)
    shift = nc.declare_dram_parameter("shift", [128, NR * 64], F32, isOutput=False)
    out = nc.declare_dram_parameter("glcm", [NIMG, N_ACC, NR, NR], F32, isOutput=True)

    with tile.TileContext(nc) as tc:
        with (
            tc.tile_pool(name="const", bufs=1) as const_pool,
            tc.tile_pool(name="strips", bufs=2) as strip_pool,
            tc.tile_pool(name="arg", bufs=3) as arg_pool,
            tc.tile_pool(name="sig", bufs=4) as sig_pool,
            tc.tile_pool(name="oub", bufs=2) as out_pool,
            tc.tile_pool(name="ps", bufs=2, space="PSUM") as psum_pool,
        ):
            # shift replicated over columns; absorbed into SBUF via one
            # copy so downstream consumers never wait on the DMA queues.
            shift_raw = const_pool.tile([128, NR, 64], F32)
            nc.sync.dma_start(
                shift_raw[:].rearrange("p a b -> p (a b)"), shift[:]
            )
            sh2 = const_pool.tile([128, NR, 64], F32)
            nc.vector.tensor_copy(
                sh2[:].rearrange("p a b -> p (a b)"),
                shift_raw[:].rearrange("p a b -> p (a b)"),
            )

            rep_ctx = (
                tc.For_i(0, loop_reps, 1) if loop_reps else contextlib.nullcontext()
            )
            with rep_ctx:
              for img in range(NIMG):
                psums = []
                for g in range(N_ACC):
                    pst = psum_pool.tile([NR, NR], F32, tag=f"ps{g}", name=f"ps{g}")
                    psums.append(pst)
                # matmuls per accumulator: strip s parity q -> 255 cols split
                acc_mm = [0] * N_ACC
                acc_total = [128, 127, 128, 127]  # ceil/floor of 255 by parity

                for si, (r0c, r0p, P) in enumerate(STRIPS):
                    svc = {}
                    for nm, r0, cc in (("c", r0c, 0), ("p", r0p, 1)):
                        s = strip_pool.tile(
                            [128, COLS], F32, tag="s" + nm, name="s" + nm
                        )
                        nc.sync.dma_start(s[:P], xs[img, r0 : r0 + P, cc : cc + COLS])
                        s2 = strip_pool.tile(
                            [128, COLS], F32, tag="s2" + nm, name="s2" + nm
                        )
                        nc.vector.tensor_copy(s2[:P], s[:P])
                        svc[nm] = s2
                    for bi, (c0, CB) in enumerate(COL_BATCHES):
                        sgs = {}
                        for nm in ("c", "p"):
                            A = arg_pool.tile([128, NR, 64], F32, tag="arg", name="A")
                            xb = (
                                svc[nm][:P, c0 : c0 + CB]
                                .unsqueeze(1)
                                .broadcast_to([P, NR, CB])
                            )
                            # A = x - shift: row 0 -> x+10, row 64 -> x-11
                            shb = (
                                sh2[:P, :, 0]
                                .unsqueeze(2)
                                .broadcast_to([P, NR, CB])
                            )
                            sub_eng = nc.vector if (bi % 2 == 0) == (nm == 'c') else nc.gpsimd
                            sub_eng.tensor_sub(A[:P, :, 0:CB], xb, shb)
                            # sig = sigmoid(640*A); rows 0/64 exactly 1/0
                            sg = sig_pool.tile(
                                [128, NR, 64], mm_dtype, tag="sg" + nm, name="sg" + nm
                            )
                            nc.scalar.activation(
                                sg[:P, :, 0:CB],
                                A[:P, :, 0:CB],
                                mybir.ActivationFunctionType.Sigmoid,
                                scale=640.0,
                            )
                            sgs[nm] = sg
                        for c in range(CB):
                            acc = 2 * si + ((c0 + c) % 2)
                            nc.tensor.matmul(
                                psums[acc][:, :],
                                sgs["c"][:P, :, c],
                                sgs["p"][:P, :, c],
                                start=(acc_mm[acc] == 0),
                                stop=(acc_mm[acc] == acc_total[acc] - 1),
                            )
                            acc_mm[acc] += 1
                # ob: [65 partitions, N_ACC, 65] — each psum copied to one slot
                ob = out_pool.tile([NR, N_ACC, NR], F32, name="ob")
                for g in range(N_ACC):
                    nc.vector.tensor_copy(ob[:, g, :], psums[g][:, :])
                nc.sync.dma_start(
                    out[img].rearrange("a r c -> r a c"), ob[:]
                )
    if split:
        _split_waits(nc)
    return nc


def _split_waits(nc):
    """This walrus build rejects >1 sync wait on ANY instruction struct
    (even Tile's own end-of-kernel drain). Rewrite every multi-wait
    instruction into a chain of single-wait same-engine drains followed
    by the instruction carrying its last wait.
    """
    n = 0
    for bb in nc.m.functions[0].blocks:
        out = []
        for ins in bb.instructions:
            si = ins.sync_info
            if si is not None and si.on_wait and len(si.on_wait) > 1:
                waits = list(si.on_wait)
                for w in waits[:-1]:
                    out.append(
                        mybir.InstDrain(
                            name=f"waitsplit-{n}",
                            engine=ins.engine,
                            sync_info=mybir.SyncInfo(on_wait=[w], on_update=[]),
                        )
                    )
                    n += 1
                ins.sync_info = mybir.SyncInfo(
                    on_wait=waits[-1:], on_update=list(si.on_update or [])
                )
            out.append(ins)
        bb.instructions[:] = out
    return n


def make_in_maps(x):
    # shift[0] = -10 (sigmoid == 1), shift[k] = k/64, shift[64] = +11
    # (sigmoid == 0 for x in [0,1))
    sv = np.arange(0, NR, dtype=np.float32) / np.float32(NG)
    sv[0] = -10.0
    sv[NG] = 11.0
    shift = np.ascontiguousarray(
        np.broadcast_to(np.repeat(sv, 64)[None, :], (128, NR * 64))
    )
    return [
        {"xs": np.ascontiguousarray(x[2 * k : 2 * k + 2]), "shift": shift}
        for k in range(8)
    ]


def _finish_host(raw):
    # raw: [16, N_ACC, NR, NR] — fp64-sum accumulators, 2D second
    # difference (the E transform on both axes), then normalize.
    s = raw.astype(np.float64).sum(axis=1)  # [16, NR, NR]
    g = s[:, :NG, :NG] - s[:, 1:, :NG] - s[:, :NG, 1:] + s[:, 1:, 1:]
    g = g / g.sum(axis=(1, 2), keepdims=True)
    return g.astype(np.float32)


_NC = None


def kernel(x, offset_r=1, offset_c=1, **_):
    global _NC
    assert int(offset_r) == 1 and int(offset_c) == 1
    x = np.ascontiguousarray(np.asarray(x, dtype=np.float32).reshape(16, H, W))
    if _NC is None:
        _NC = _build_program()
    res = run_bass_kernel_spmd(_NC, make_in_maps(x), core_ids=list(range(8)))
    raw = np.concatenate([r["glcm"] for r in res.results], axis=0)
    return _finish_host(raw).reshape(16, 1, NG, NG, 1)


if __name__ == "__main__":
    _build_program()
    print("build OK")
